# revision 1
# baseline (speedup 1.0000x reference)
"""Trainium2 Bass kernel for NeuralECMModel (gnn_message_passing).

Math (per node n):
  ent  = entity_emb @ Wp.T + bp                                   [N,50]
  node = einsum('ni,oij,nj->no', q, Wbil, ent) + bbil             [N,50]
  wtext= sum_k s[n,k]*nbr[n,k,:] + s[n,63]*node[n,:]              [N,50]
  agg  = wtext @ Wg.T                                             [N,50]
  out  = elu(agg + g_bias) @ Wr.T + br                            [N,1]

Key restructuring (vs naive): Wg is folded into both branches so `agg`
is accumulated directly in PSUM by the PE:
  agg[p,n] = sum_{(d,k)} Wg[p,d]*s[n,k]*nbr[n,k,d]        (PE contraction
             over 25 chunks of the transposed neighbor stream)
           + s63[n]*(q Wtil[p] ent + bbilg[p])            (row-major bilinear,
             transpose-matmul-accumulated into the same PSUM tile)
  with Wtil[p,i,j] = sum_o Wg[p,o]*Wbil[o,i,j], bbilg = Wg @ bbil.

This removes the k-tree reduction from the vector engines entirely; the
score multiply is ONE full-rate bf16 DVE op per 500-node macro tile.

Sharding: pure data parallel over nodes, N=20000 -> 2500 nodes/core x 8.
"""

import numpy as np
import ml_dtypes

import concourse.bass as bass
import concourse.bacc as bacc
import concourse.tile as tile
import concourse.mybir as mybir
from concourse.bass_utils import run_bass_kernel_spmd
from concourse.masks import make_identity

F32 = mybir.dt.float32
BF16 = mybir.dt.bfloat16
OP = mybir.AluOpType
AF = mybir.ActivationFunctionType
AX = mybir.AxisListType

N_CORES = 8
N = 20000
NLOC = N // N_CORES   # 2500
K = 63
D = 50
E = 300
EA = 304              # padded augmented entity rows (300 + ones + 3 zero)
P = 128
SR = 125              # bilinear sub-tile rows
# macro tile sizes: small ramp-up/ramp-down tiles shorten the DMA-bound
# startup and the drain at the end
MTS = (250, 250, 500, 500, 250, 500, 250)
N_MT = len(MTS)
NCH = 25              # neighbor (d,k) chunks of 126 rows
NSPLIT = (8, 8, 9)    # neighbor chunk-group sizes (finer DMA/compute overlap)
CR = 2 * K            # 126 rows per chunk (2 d's x 63 k's)
OJ = D * D            # 2500

# bilinear o-split per sub-tile: DVE takes o in [0, osp), Pool [osp, D);
# alternating 30/40 averages 35 despite the 10-o egress-chunk granularity
OSPS = (30, 40)
# U psum chunk width in o's (each *D wide); one PSUM bank per chunk so every
# matmul output is bank-aligned (mid-bank matmul writes corrupt silently)
UW = 10
UCH = (10, 10, 10, 10, 10)

_CACHE = {}


def _tree_levels(w):
    """Pairwise-halving splits: [(hi, lo), ...] meaning x[0:hi] += x[lo:lo+hi]."""
    out = []
    while w > 1:
        lo = (w + 1) // 2
        hi = w - lo
        out.append((hi, lo))
        w = lo
    return out


def build_program(br_val: float, skip_bil=False, skip_nbr=False, dump_ng=False):
    nc = bacc.Bacc("TRN2", debug=False, num_devices=N_CORES)

    # ---- per-core inputs ----
    t_nbrT = nc.dram_tensor("nbrT", [NCH * CR, NLOC], BF16, kind="ExternalInput")
    # f32 pack: cols 0..19 s63 (col per sub-tile), col 20 rows 0..49 = g_bias
    t_f32p = nc.dram_tensor("f32p", [SR, NLOC // SR + 1], F32, kind="ExternalInput")
    # [50, 5000]: qT | W2til side by side
    t_qW = nc.dram_tensor("qW", [D, NLOC + OJ], BF16, kind="ExternalInput")
    t_entT0 = nc.dram_tensor("entT0", [128, NLOC], BF16, kind="ExternalInput")
    t_entT1 = nc.dram_tensor("entT1", [128, NLOC], BF16, kind="ExternalInput")
    t_entT2 = nc.dram_tensor("entT2", [EA - 256, NLOC], BF16, kind="ExternalInput")
    # [128, 201]: WpT0 | WpT1 | bbilg_rep+WrT | WpT2 (rows 0..47)
    t_wpack = nc.dram_tensor("wpack", [128, 100 + D + 1 + D], BF16, kind="ExternalInput")
    # [126, 3750]: sTrep | WgK
    t_sWgK = nc.dram_tensor("sWgK", [CR, NLOC + NCH * D], BF16, kind="ExternalInput")
    t_s63r = nc.dram_tensor("s63r", [1, NLOC], BF16, kind="ExternalInput")
    t_out = nc.dram_tensor("out", [1, NLOC], F32, kind="ExternalOutput")
    t_dbg = (
        nc.dram_tensor("dbg", [SR, (NLOC // SR) * D], BF16, kind="ExternalOutput")
        if dump_ng else None
    )

    with tile.TileContext(nc) as tc:
        with (
            tc.tile_pool(name="res", bufs=1) as res,
            tc.tile_pool(name="nbrp", bufs=2) as nbrp,
            tc.tile_pool(name="snbrp", bufs=2) as snbrp,
            tc.tile_pool(name="usbd", bufs=3) as usbd_p,
            tc.tile_pool(name="usbg", bufs=3) as usbg_p,
            tc.tile_pool(name="prodd", bufs=3) as prodd_p,
            tc.tile_pool(name="prodg", bufs=3) as prodg_p,
            tc.tile_pool(name="small", bufs=8) as small,
            tc.tile_pool(name="tailp", bufs=3) as tailp,
            tc.tile_pool(name="ps_agg", bufs=2, space="PSUM") as ps_agg,
            tc.tile_pool(name="ps_ent", bufs=1, space="PSUM") as ps_ent,
            tc.tile_pool(name="ps_u", bufs=4, space="PSUM") as ps_u,
            tc.tile_pool(name="ps_o", bufs=1, space="PSUM") as ps_o,
            nc.allow_low_precision(reason="bf16 tree-reduce; node term is ~10% of signal"),
        ):
            # ---- residents: bilinear-path deps first so PE/ACT/DVE can start
            # the sub-tile pipeline while the big neighbor stream lands.
            qW_sb = res.tile([D, NLOC + OJ], BF16)
            nc.sync.dma_start(out=qW_sb, in_=t_qW[:])
            wpack_sb = res.tile([P, 100 + D + 1 + D], BF16)
            nc.sync.dma_start(out=wpack_sb, in_=t_wpack[:])
            f32p_sb = res.tile([SR, NLOC // SR + 1], F32)
            nc.sync.dma_start(out=f32p_sb, in_=t_f32p[:])
            entT_sb = [
                res.tile([128, NLOC], BF16, tag="entT0", name="entT0_sb"),
                res.tile([128, NLOC], BF16, tag="entT1", name="entT1_sb"),
                res.tile([EA - 256, NLOC], BF16, tag="entT2", name="entT2_sb"),
            ]
            for sb, t in zip(entT_sb, (t_entT0, t_entT1, t_entT2)):
                nc.sync.dma_start(out=sb, in_=t[:])
            nbrT_v = t_nbrT[:].rearrange("(c p) n -> p c n", p=CR)

            nbr0_parts = []
            c0 = 0
            for gi, gw in enumerate(NSPLIT):
                t = nbrp.tile(
                    [CR, gw * MTS[0]], BF16, tag=f"nbr{gi}", name=f"nbr0_{gi}"
                )
                nc.sync.dma_start(
                    out=t.rearrange("p (c n) -> p c n", c=gw),
                    in_=nbrT_v[:, c0 : c0 + gw, 0 : MTS[0]],
                )
                nbr0_parts.append(t)
                c0 += gw
            sWgK_sb = res.tile([CR, NLOC + NCH * D], BF16)
            nc.sync.dma_start(out=sWgK_sb, in_=t_sWgK[:])
            s63r_sb = res.tile([1, NLOC], BF16)
            nc.sync.dma_start(out=s63r_sb, in_=t_s63r[:])
            ident_sb = res.tile([P, P], BF16)
            make_identity(nc, ident_sb)
            out_row = res.tile([1, NLOC], F32)

            sTrep_sb = sWgK_sb[:, 0:NLOC]
            WgK_sb = sWgK_sb[:, NLOC : NLOC + NCH * D]

            qT_sb = qW_sb[:, 0:NLOC]
            W2til_sb = qW_sb[:, NLOC : NLOC + OJ]
            WpT_sb = [
                wpack_sb[:, 0:D],
                wpack_sb[:, D : 2 * D],
                wpack_sb[0 : EA - 256, 151 : 151 + D],
            ]
            bbilg_rep = wpack_sb[:, 100:150]          # [128, 50]
            WrT_col = wpack_sb[0:D, 150:151]          # [50, 1]
            s63p_sb = f32p_sb[:, 0 : NLOC // SR]
            gb_sb = f32p_sb[0:D, NLOC // SR : NLOC // SR + 1]

            n0 = 0
            gsub = 0
            for mt in range(N_MT):
                ntx = MTS[mt]
                ns = slice(n0, n0 + ntx)

                if mt == 0:
                    nbr_parts = nbr0_parts
                else:
                    nbr_parts = []
                    c0 = 0
                    for gi, gw in enumerate(NSPLIT):
                        t = nbrp.tile(
                            [CR, gw * ntx], BF16, tag=f"nbr{gi}", name=f"nbr{mt}_{gi}"
                        )
                        nc.sync.dma_start(
                            out=t.rearrange("p (c n) -> p c n", c=gw),
                            in_=nbrT_v[:, c0 : c0 + gw, ns],
                        )
                        nbr_parts.append(t)
                        c0 += gw
                aggC = ps_agg.tile([D, ntx], F32, tag="aggC", name="aggC")

                def emit_snbr():
                    views = []
                    for gi, gw in enumerate(NSPLIT):
                        st_ = snbrp.tile(
                            [CR, gw * ntx], BF16, tag=f"snbr{gi}", name=f"snbr{gi}"
                        )
                        sv = st_.rearrange("p (c n) -> p c n", c=gw)
                        sT_bx = sTrep_sb[:, ns].unsqueeze(1).broadcast_to(
                            [CR, gw, ntx]
                        )
                        nc.vector.tensor_mul(
                            sv, nbr_parts[gi].rearrange("p (c n) -> p c n", c=gw), sT_bx
                        )
                        views.append(sv)
                    return views

                # scores multiply: ONE full-rate bf16 DVE op.  For mt 0 it is
                # emitted after the bilinear subs so DVE starts on work whose
                # inputs land first (the big neighbor DMA is still in flight).
                snbr_v = None
                if not skip_nbr and mt > 0:
                    snbr_v = emit_snbr()  # (sA, sB)

                # ---- bilinear per 125-row sub-tile (row-major) ----
                node_gbs = []
                nsub = ntx // SR
                for st in range(nsub if not skip_bil else 0):
                    r0 = n0 + st * SR
                    rs = slice(r0, r0 + SR)
                    col = gsub + st

                    ent_ps = ps_ent.tile([SR, D], F32)
                    for c in range(3):
                        nc.tensor.matmul(
                            ent_ps,
                            entT_sb[c][:, rs],
                            WpT_sb[c],
                            start=(c == 0),
                            stop=(c == 2),
                        )
                    # PSUM->SBUF with the s63 per-node scale folded in
                    ents = small.tile([SR, D], BF16, tag="ents")
                    nc.scalar.activation(
                        out=ents,
                        in_=ent_ps,
                        func=AF.Copy,
                        scale=s63p_sb[:, col : col + 1],
                    )

                    # U = q @ W2til (chunks along o), egress to bf16 SBUF.
                    # DVE owns o in [0, osp), Pool owns [osp, D). Separate
                    # tiles per engine so their buffer rotations never couple.
                    osp = OSPS[(gsub + st) % len(OSPS)]
                    usbd = usbd_p.tile([SR, 40 * D], BF16)
                    usbg = usbg_p.tile([SR, 20 * D], BF16)
                    for off in range(0, D, UW):
                        w = UW
                        u_ps = ps_u.tile([SR, w * D], F32)
                        nc.tensor.matmul(
                            u_ps,
                            qT_sb[:, rs],
                            W2til_sb[:, off * D : (off + w) * D],
                            start=True,
                            stop=True,
                        )
                        if off < osp:
                            nc.scalar.copy(
                                out=usbd[:, off * D : (off + w) * D], in_=u_ps
                            )
                        else:
                            nc.scalar.copy(
                                out=usbg[:, (off - osp) * D : (off - osp + w) * D],
                                in_=u_ps,
                            )

                    # prod = U * ents (broadcast over o); j-tree -> node_g
                    prodd = prodd_p.tile([SR, 40 * D], BF16)
                    prodg = prodg_p.tile([SR, 20 * D], BF16)
                    node_g = small.tile([SR, D], BF16, tag="node_g")
                    ev = ents.unsqueeze(1).broadcast_to([SR, D, D])
                    for eng, prod, usb, o0, o1 in (
                        (nc.vector, prodd, usbd, 0, osp),
                        (nc.gpsimd, prodg, usbg, osp, D),
                    ):
                        ow = o1 - o0
                        pvx = prod[:, 0 : ow * D].rearrange("p (o j) -> p o j", o=ow)
                        uvx = usb[:, 0 : ow * D].rearrange("p (o j) -> p o j", o=ow)
                        eng.tensor_mul(pvx, uvx, ev[:, o0:o1])
                        w = D
                        for hi, lo in _tree_levels(D):
                            if w == 2:
                                eng.tensor_add(
                                    node_g[:, o0:o1].unsqueeze(2),
                                    pvx[:, :, 0:1],
                                    pvx[:, :, 1:2],
                                )
                            else:
                                eng.tensor_add(
                                    pvx[:, :, 0:hi],
                                    pvx[:, :, 0:hi],
                                    pvx[:, :, lo : lo + hi],
                                )
                            w = lo

                    if dump_ng:
                        nc.sync.dma_start(
                            out=t_dbg[:, col * D : (col + 1) * D], in_=node_g
                        )
                    node_gbs.append(node_g)

                if not skip_nbr and snbr_v is None:
                    snbr_v = emit_snbr()

                # ---- neighbor contraction: agg[p, n] over 25 (d,k)-chunks ----
                for c in range(NCH if not skip_nbr else 0):
                    gi, cc = 0, c
                    while cc >= NSPLIT[gi]:
                        cc -= NSPLIT[gi]
                        gi += 1
                    nc.tensor.matmul(
                        aggC,
                        WgK_sb[:, c * D : (c + 1) * D],
                        snbr_v[gi][:, cc, :],
                        start=(c == 0),
                        stop=False,
                    )
                # rank-1 bias term: agg += bbilg[o] * s63[n]
                nc.tensor.matmul(
                    aggC,
                    bbilg_rep[0:1],
                    s63r_sb[:, ns],
                    start=skip_nbr,
                    stop=skip_bil,
                )
                # ---- node contribution: transpose-accumulate ----
                for st in range(nsub if not skip_bil else 0):
                    nc.tensor.matmul(
                        aggC[:, st * SR : (st + 1) * SR],
                        node_gbs[st],
                        ident_sb[0:SR, 0:SR],
                        start=False,
                        stop=True,
                    )

                # ---- tail: elu(agg+gb) @ Wr + br ----
                e_sb = tailp.tile([D, ntx], BF16, tag="e")
                nc.scalar.activation(out=e_sb, in_=aggC, func=AF.Exp, bias=gb_sb)
                r_sb = tailp.tile([D, ntx], BF16, tag="r")
                nc.scalar.activation(out=r_sb, in_=aggC, func=AF.Relu, bias=gb_sb)
                # elu(x) = min(exp(x)-1, relu(x)) exactly
                feats = tailp.tile([D, ntx], BF16, tag="feats")
                nc.vector.scalar_tensor_tensor(
                    out=feats, in0=e_sb, scalar=-1.0, in1=r_sb,
                    op0=OP.add, op1=OP.min,
                )
                out_ps = ps_o.tile([1, ntx], F32, tag="out_ps", name="out_ps")
                nc.tensor.matmul(out_ps, WrT_col, feats, start=True, stop=True)
                nc.scalar.activation(
                    out=out_row[:, ns], in_=out_ps, func=AF.Identity, bias=br_val
                )
                n0 += ntx
                gsub += nsub

            nc.sync.dma_start(out=t_out[:], in_=out_row)

    nc.finalize()
    return nc


def kernel(
    query_emb,
    entity_emb,
    neighbor_embs,
    neighbor_scores,
    Wp,
    bp,
    Wbil,
    bbil,
    Wg,
    g_bias,
    Wr,
    br,
):
    br_val = float(np.asarray(br).reshape(-1)[0])
    if "nc" not in _CACHE:
        _CACHE["nc"] = build_program(br_val)
    nc = _CACHE["nc"]

    bf = ml_dtypes.bfloat16
    q = np.asarray(query_emb, np.float32)
    ent = np.asarray(entity_emb, np.float32)
    nbr = np.asarray(neighbor_embs, np.float32)
    sc = np.asarray(neighbor_scores, np.float32)
    Wg_ = np.asarray(Wg, np.float32)
    Wbil_ = np.asarray(Wbil, np.float32)

    # ---- shared weight prep ----
    # Wtil[p,i,j] = sum_o Wg[p,o] Wbil[o,i,j]; W2til[i, p*D+j] = Wtil[p,i,j]
    Wtil = np.einsum("po,oij->pij", Wg_, Wbil_)
    W2til_f = Wtil.transpose(1, 0, 2).reshape(D, OJ)
    bbilg = Wg_ @ np.asarray(bbil, np.float32)  # [50]
    WpT_aug = np.zeros((EA, D), np.float32)
    WpT_aug[0:E] = np.asarray(Wp, np.float32).T
    WpT_aug[E] = np.asarray(bp, np.float32)
    # WgK[(db,k), c*D+o] = Wg[o, 2c+db]
    WgT = Wg_.T  # [d, o]
    WgK = np.empty((CR, NCH * D), np.float32)
    for c in range(NCH):
        WgK[:, c * D : (c + 1) * D] = np.repeat(WgT[2 * c : 2 * c + 2], K, axis=0)
    # wpack: WpT0 | WpT1 | bbilg_rep+WrT | WpT2
    wpack = np.zeros((P, 100 + D + 1 + D), np.float32)
    wpack[:, 0:D] = WpT_aug[0:128]
    wpack[:, D : 2 * D] = WpT_aug[128:256]
    wpack[:, 100:150] = bbilg[None, :]
    wpack[0:D, 150] = np.asarray(Wr, np.float32).reshape(-1)
    wpack[0 : EA - 256, 151 : 151 + D] = WpT_aug[256:EA]
    wpack = wpack.astype(bf)
    gb = np.asarray(g_bias, np.float32)

    in_maps = []
    for c in range(N_CORES):
        s = slice(c * NLOC, (c + 1) * NLOC)
        ent_aug = np.zeros((EA, NLOC), np.float32)
        ent_aug[0:E] = ent[s].T
        ent_aug[E] = 1.0
        nbrT = nbr[s].transpose(2, 1, 0).reshape(NCH * CR, NLOC)
        sT = sc[s, 0:K].T  # [63, NLOC]
        f32p = np.zeros((SR, NLOC // SR + 1), np.float32)
        f32p[:, 0 : NLOC // SR] = sc[s, K].reshape(NLOC // SR, SR).T
        f32p[0:D, NLOC // SR] = gb
        s63r = sc[s, K][None, :]  # [1, NLOC]
        in_maps.append(
            {
                "nbrT": np.ascontiguousarray(nbrT).astype(bf),
                "sWgK": np.ascontiguousarray(
                    np.concatenate(
                        [np.concatenate([sT, sT], axis=0), WgK], axis=1
                    )
                ).astype(bf),
                "f32p": np.ascontiguousarray(f32p),
                "s63r": np.ascontiguousarray(s63r).astype(bf),
                "qW": np.ascontiguousarray(
                    np.concatenate([q[s].T, W2til_f], axis=1)
                ).astype(bf),
                "entT0": np.ascontiguousarray(ent_aug[0:128]).astype(bf),
                "entT1": np.ascontiguousarray(ent_aug[128:256]).astype(bf),
                "entT2": np.ascontiguousarray(ent_aug[256:EA]).astype(bf),
                "wpack": wpack,
            }
        )

    _CACHE["last_in_maps"] = in_maps
    res = run_bass_kernel_spmd(nc, in_maps, core_ids=list(range(N_CORES)))
    out = np.concatenate(
        [res.results[c]["out"].reshape(NLOC, 1) for c in range(N_CORES)], axis=0
    )
    return out.astype(np.float32)



# revision 12
# speedup vs baseline: 133.0443x; 133.0443x over previous
"""Trainium2 Bass kernel for NeuralECMModel (gnn_message_passing).

Math (per node n):
  ent  = entity_emb @ Wp.T + bp                                   [N,50]
  node = einsum('ni,oij,nj->no', q, Wbil, ent) + bbil             [N,50]
  wtext= sum_k s[n,k]*nbr[n,k,:] + s[n,63]*node[n,:]              [N,50]
  agg  = wtext @ Wg.T                                             [N,50]
  out  = elu(agg + g_bias) @ Wr.T + br                            [N,1]

Key restructuring (vs naive): Wg is folded into both branches so `agg`
is accumulated directly in PSUM by the PE:
  agg[p,n] = sum_{(d,k)} Wg[p,d]*s[n,k]*nbr[n,k,d]        (PE contraction
             over 25 chunks of the transposed neighbor stream)
           + s63[n]*(q Wtil[p] ent + bbilg[p])            (row-major bilinear,
             transpose-matmul-accumulated into the same PSUM tile)
  with Wtil[p,i,j] = sum_o Wg[p,o]*Wbil[o,i,j], bbilg = Wg @ bbil.

This removes the k-tree reduction from the vector engines entirely; the
score multiply is ONE full-rate bf16 DVE op per 500-node macro tile.

Sharding: pure data parallel over nodes, N=20000 -> 2500 nodes/core x 8.
"""

import numpy as np
import ml_dtypes

import concourse.bass as bass
import concourse.bacc as bacc
import concourse.tile as tile
import concourse.mybir as mybir
from concourse.bass_utils import run_bass_kernel_spmd
from concourse.masks import make_identity

F32 = mybir.dt.float32
BF16 = mybir.dt.bfloat16
OP = mybir.AluOpType
AF = mybir.ActivationFunctionType
AX = mybir.AxisListType

N_CORES = 8
N = 20000
NLOC = N // N_CORES   # 2500
K = 63
D = 50
E = 300
EA = 304              # padded augmented entity rows (300 + ones + 3 zero)
P = 128
SR = 125              # bilinear sub-tile rows
NCH = 25              # neighbor (d,k) chunks of 126 rows
CR = 2 * K            # 126 rows per chunk (2 d's x 63 k's)
OJ = D * D            # 2500

RW = 500              # agg region width (one PSUM bank: 500 f32)
# per-sub-tile U o-chunk assignment (n_direct_psum_dve, n_sbuf_dve, n_pool)
UPLAN = ((0, 5, 0),)   # all U chunks: ACT egress copy -> DVE mul+reduce
# snbr chunk engine: True -> Pool, False -> DVE
SNBR_POOL = tuple(False for _ in range(25))
# U psum chunk width in o's (each *D wide); one PSUM bank per chunk so every
# matmul output is bank-aligned
UW = 10

_CACHE = {}


def _tree_levels(w):
    """Pairwise-halving splits: [(hi, lo), ...] meaning x[0:hi] += x[lo:lo+hi]."""
    out = []
    while w > 1:
        lo = (w + 1) // 2
        hi = w - lo
        out.append((hi, lo))
        w = lo
    return out


def build_program(br_val: float, skip_bil=False, skip_nbr=False, dump_ng=False,
                  repeat=1, uplan=None, dve_tree=False, snbr_pool=None):
    """repeat>1 wraps the whole per-run body (including all input streaming)
    in a hardware loop - used by test.py to time steady-state per-execution
    HW time with the ~3.6ms axon launch round-trip amortized away.

    v2 structure (single pass over all NLOC nodes):
      - neighbor stream DMA'd chunk-major: 25 chunks of [126, 2500] with 5KB
        contiguous rows (vs 0.5-1KB rows of the macro-tile slicing).
      - agg accumulated in 5 persistent PSUM bank regions [50, 500] covering
        all 2500 nodes; ONE tail (elu + Wr head) at the end.
      - bilinear j-reduction on DVE via a single tensor_reduce(X) instead of
        a 6-instruction pairwise tree; Pool keeps the tree for its o-share.
      - per-sub-tile engine assignment of the 5 U o-chunks: chunk 0 is
        consumed by DVE straight from PSUM (saves the ACT egress copy),
        the rest split DVE/Pool via an alternating pattern.
    """
    nc = bacc.Bacc("TRN2", debug=False, num_devices=N_CORES)

    # ---- per-core inputs (layouts unchanged from v1) ----
    t_nbrT = nc.dram_tensor("nbrT", [NCH * CR, NLOC], BF16, kind="ExternalInput")
    t_f32p = nc.dram_tensor("f32p", [SR, NLOC // SR + 1], F32, kind="ExternalInput")
    t_qW = nc.dram_tensor("qW", [D, NLOC + OJ], BF16, kind="ExternalInput")
    t_entT0 = nc.dram_tensor("entT0", [128, NLOC], BF16, kind="ExternalInput")
    t_entT1 = nc.dram_tensor("entT1", [128, NLOC], BF16, kind="ExternalInput")
    t_entT2 = nc.dram_tensor("entT2", [EA - 256, NLOC], BF16, kind="ExternalInput")
    t_wpack = nc.dram_tensor("wpack", [128, 100 + D + 1 + D], BF16, kind="ExternalInput")
    t_sWgK = nc.dram_tensor("sWgK", [CR, NLOC + NCH * D], BF16, kind="ExternalInput")
    t_s63r = nc.dram_tensor("s63r", [1, NLOC], BF16, kind="ExternalInput")
    t_out = nc.dram_tensor("out", [1, NLOC], F32, kind="ExternalOutput")

    NREG = NLOC // RW          # 5 agg regions of 500 nodes (1 PSUM bank each)
    NSUB = NLOC // SR          # 20 bilinear sub-tiles
    SPR = RW // SR             # 4 sub-tiles per region

    with tile.TileContext(nc) as tc:
        with (
            tc.tile_pool(name="res", bufs=1) as res,
            tc.tile_pool(name="nbrp", bufs=4) as nbrp,
            tc.tile_pool(name="snbrp", bufs=3) as snbrp,
            tc.tile_pool(name="usbd", bufs=3) as usbd_p,
            tc.tile_pool(name="usbg", bufs=3) as usbg_p,
            tc.tile_pool(name="prodd", bufs=3) as prodd_p,
            tc.tile_pool(name="prodg", bufs=3) as prodg_p,
            tc.tile_pool(name="small", bufs=8) as small,
            tc.tile_pool(name="nodegs", bufs=20) as nodegs,
            tc.tile_pool(name="tailp", bufs=3) as tailp,
            tc.tile_pool(name="ps_agg", bufs=1, space="PSUM") as ps_agg,
            tc.tile_pool(name="ps_u", bufs=2, space="PSUM") as ps_u,
            tc.tile_pool(name="ps_misc", bufs=1, space="PSUM") as ps_misc,
            nc.allow_low_precision(reason="bf16 bilinear reduce; node term is ~10% of signal"),
        ):
            ident_sb = res.tile([P, P], BF16)
            make_identity(nc, ident_sb)

            def emit_body():
                # ---- residents ----
                qW_sb = res.tile([D, NLOC + OJ], BF16)
                nc.sync.dma_start(out=qW_sb, in_=t_qW[:])
                wpack_sb = res.tile([P, 100 + D + 1 + D], BF16)
                nc.sync.dma_start(out=wpack_sb, in_=t_wpack[:])
                f32p_sb = res.tile([SR, NLOC // SR + 1], F32)
                nc.sync.dma_start(out=f32p_sb, in_=t_f32p[:])
                entT_sb = [
                    res.tile([128, NLOC], BF16, tag="entT0", name="entT0_sb"),
                    res.tile([128, NLOC], BF16, tag="entT1", name="entT1_sb"),
                    res.tile([EA - 256, NLOC], BF16, tag="entT2", name="entT2_sb"),
                ]
                for sb, t in zip(entT_sb, (t_entT0, t_entT1, t_entT2)):
                    nc.sync.dma_start(out=sb, in_=t[:])
                sWgK_sb = res.tile([CR, NLOC + NCH * D], BF16)
                nc.sync.dma_start(out=sWgK_sb, in_=t_sWgK[:])
                s63r_sb = res.tile([1, NLOC], BF16)
                nc.sync.dma_start(out=s63r_sb, in_=t_s63r[:])
                out_row = res.tile([1, NLOC], F32)

                sTrep_sb = sWgK_sb[:, 0:NLOC]
                WgK_sb = sWgK_sb[:, NLOC : NLOC + NCH * D]
                qT_sb = qW_sb[:, 0:NLOC]
                W2til_sb = qW_sb[:, NLOC : NLOC + OJ]
                WpT_sb = [
                    wpack_sb[:, 0:D],
                    wpack_sb[:, D : 2 * D],
                    wpack_sb[0 : EA - 256, 151 : 151 + D],
                ]
                bbilg_rep = wpack_sb[:, 100:150]
                WrT_col = wpack_sb[0:D, 150:151]
                s63p_sb = f32p_sb[:, 0 : NLOC // SR]
                gb_sb = f32p_sb[0:D, NLOC // SR : NLOC // SR + 1]

                # ---- neighbor chunk DMAs (self-throttled by pool bufs) ----
                nbr_c = []
                for c in range(NCH if not skip_nbr else 0):
                    t = nbrp.tile([CR, NLOC], BF16, tag="nbr", name=f"nbr{c}")
                    nc.sync.dma_start(out=t, in_=t_nbrT[c * CR : (c + 1) * CR, :])
                    nbr_c.append(t)

                # persistent agg regions
                aggR = [
                    ps_agg.tile([D, RW], F32, tag=f"agg{r}", name=f"agg{r}")
                    for r in range(NREG)
                ]

                def emit_snbr(c):
                    st_ = snbrp.tile([CR, NLOC], BF16, tag="snbr", name=f"snbr{c}")
                    sp = snbr_pool if snbr_pool is not None else SNBR_POOL
                    eng = nc.gpsimd if sp[c] else nc.vector
                    eng.tensor_mul(st_, nbr_c[c], sTrep_sb)
                    return st_

                def emit_nbr_mm(c, snbr_t):
                    for r in range(NREG):
                        nc.tensor.matmul(
                            aggR[r],
                            WgK_sb[:, c * D : (c + 1) * D],
                            snbr_t[:, r * RW : (r + 1) * RW],
                            start=(c == 0),
                            stop=False,
                        )

                # ---- interleaved bilinear sub-tiles + neighbor chunks ----
                ci = 0  # next neighbor chunk to emit
                node_gbs = []

                def emit_chunk():
                    nonlocal ci
                    if not skip_nbr and ci < NCH:
                        emit_nbr_mm(ci, emit_snbr(ci))
                        ci += 1

                for st in range(NSUB if not skip_bil else 0):
                    r0 = st * SR
                    rs = slice(r0, r0 + SR)

                    ent_ps = ps_misc.tile([SR, D], F32, tag="misc", name="ent_ps")
                    for c in range(3):
                        nc.tensor.matmul(
                            ent_ps,
                            entT_sb[c][:, rs],
                            WpT_sb[c],
                            start=(c == 0),
                            stop=(c == 2),
                        )
                    ents = small.tile([SR, D], BF16, tag="ents")
                    nc.scalar.activation(
                        out=ents, in_=ent_ps, func=AF.Copy,
                        scale=s63p_sb[:, st : st + 1],
                    )
                    ev = ents.unsqueeze(1).broadcast_to([SR, D, D])

                    # U chunk plan for this sub-tile: (n_direct, n_dve, n_pool)
                    _up = uplan if uplan is not None else UPLAN
                    nD, nV, nG = _up[st % len(_up)]
                    node_g = nodegs.tile([SR, D], BF16, tag="node_g",
                                         name=f"node_g{st}")
                    node_gbs.append(node_g)

                    # direct-PSUM chunks: DVE consumes u_ps f32 (no ACT copy)
                    for k in range(nD):
                        o0 = k * UW
                        u_ps = ps_u.tile([SR, UW * D], F32, tag="ups")
                        nc.tensor.matmul(
                            u_ps, qT_sb[:, rs],
                            W2til_sb[:, o0 * D : (o0 + UW) * D],
                            start=True, stop=True,
                        )
                        prodd = prodd_p.tile([SR, UW * D], BF16, tag="prodD")
                        pv = prodd.rearrange("p (o j) -> p o j", o=UW)
                        nc.vector.tensor_mul(
                            pv, u_ps.rearrange("p (o j) -> p o j", o=UW),
                            ev[:, o0 : o0 + UW],
                        )
                        nc.vector.tensor_reduce(
                            node_g[:, o0 : o0 + UW].unsqueeze(2), pv,
                            axis=AX.X, op=OP.add,
                        )

                    # DVE-via-SBUF chunks (ACT egress copy, batched mul+reduce)
                    oV = nD * UW
                    wV = nV * UW
                    if nV:
                        usbd = usbd_p.tile([SR, wV * D], BF16, tag="usbd")
                        for k in range(nV):
                            o0 = (nD + k) * UW
                            u_ps = ps_u.tile([SR, UW * D], F32, tag="ups")
                            nc.tensor.matmul(
                                u_ps, qT_sb[:, rs],
                                W2til_sb[:, o0 * D : (o0 + UW) * D],
                                start=True, stop=True,
                            )
                            nc.scalar.copy(
                                out=usbd[:, k * UW * D : (k + 1) * UW * D],
                                in_=u_ps,
                            )
                        prodv = prodd_p.tile([SR, wV * D], BF16, tag="prodV")
                        pv = prodv.rearrange("p (o j) -> p o j", o=wV)
                        nc.vector.tensor_mul(
                            pv, usbd.rearrange("p (o j) -> p o j", o=wV),
                            ev[:, oV : oV + wV],
                        )
                        if dve_tree:
                            w = D
                            for hi, lo in _tree_levels(D):
                                if w == 2:
                                    nc.vector.tensor_add(
                                        node_g[:, oV : oV + wV].unsqueeze(2),
                                        pv[:, :, 0:1], pv[:, :, 1:2],
                                    )
                                else:
                                    nc.vector.tensor_add(
                                        pv[:, :, 0:hi], pv[:, :, 0:hi],
                                        pv[:, :, lo : lo + hi],
                                    )
                                w = lo
                        else:
                            nc.vector.tensor_reduce(
                                node_g[:, oV : oV + wV].unsqueeze(2), pv,
                                axis=AX.X, op=OP.add,
                            )

                    # Pool chunks (ACT egress copy, mul + pairwise tree)
                    oG = (nD + nV) * UW
                    wG = nG * UW
                    if nG:
                        usbg = usbg_p.tile([SR, wG * D], BF16, tag="usbg")
                        for k in range(nG):
                            o0 = (nD + nV + k) * UW
                            u_ps = ps_u.tile([SR, UW * D], F32, tag="ups")
                            nc.tensor.matmul(
                                u_ps, qT_sb[:, rs],
                                W2til_sb[:, o0 * D : (o0 + UW) * D],
                                start=True, stop=True,
                            )
                            nc.scalar.copy(
                                out=usbg[:, k * UW * D : (k + 1) * UW * D],
                                in_=u_ps,
                            )
                        prodg = prodg_p.tile([SR, wG * D], BF16, tag="prodG")
                        pg = prodg.rearrange("p (o j) -> p o j", o=wG)
                        nc.gpsimd.tensor_mul(
                            pg, usbg.rearrange("p (o j) -> p o j", o=wG),
                            ev[:, oG : oG + wG],
                        )
                        w = D
                        for hi, lo in _tree_levels(D):
                            if w == 2:
                                nc.gpsimd.tensor_add(
                                    node_g[:, oG : oG + wG].unsqueeze(2),
                                    pg[:, :, 0:1], pg[:, :, 1:2],
                                )
                            else:
                                nc.gpsimd.tensor_add(
                                    pg[:, :, 0:hi], pg[:, :, 0:hi],
                                    pg[:, :, lo : lo + hi],
                                )
                            w = lo

                    # keep the neighbor pipeline fed
                    emit_chunk()
                    if st % 2 == 1:
                        emit_chunk()

                while ci < NCH and not skip_nbr:
                    emit_chunk()

                # node contribution -> agg (transpose-accumulate), deferred
                # so chunk 0's start=True full-region write is always first
                for st, ng in enumerate(node_gbs):
                    r = st // SPR
                    nc.tensor.matmul(
                        aggR[r][:, (st % SPR) * SR : (st % SPR + 1) * SR],
                        ng,
                        ident_sb[0:SR, 0:SR],
                        start=(skip_nbr and st % SPR == 0),
                        stop=False,
                    )

                # rank-1 bias term closes each region's accumulation group
                for r in range(NREG):
                    nc.tensor.matmul(
                        aggR[r],
                        bbilg_rep[0:1],
                        s63r_sb[:, r * RW : (r + 1) * RW],
                        start=(skip_nbr and skip_bil),
                        stop=True,
                    )

                # ---- tail: elu(agg+gb) @ Wr + br, one pass per region ----
                for r in range(NREG):
                    e_sb = tailp.tile([D, RW], BF16, tag="e")
                    nc.scalar.activation(out=e_sb, in_=aggR[r], func=AF.Exp,
                                         bias=gb_sb)
                    r_sb = tailp.tile([D, RW], BF16, tag="r")
                    nc.scalar.activation(out=r_sb, in_=aggR[r], func=AF.Relu,
                                         bias=gb_sb)
                    feats = tailp.tile([D, RW], BF16, tag="feats")
                    nc.vector.scalar_tensor_tensor(
                        out=feats, in0=e_sb, scalar=-1.0, in1=r_sb,
                        op0=OP.add, op1=OP.min,
                    )
                    out_ps = ps_misc.tile([1, RW], F32, tag="misc", name="out_ps")
                    nc.tensor.matmul(out_ps, WrT_col, feats, start=True, stop=True)
                    nc.scalar.activation(
                        out=out_row[:, r * RW : (r + 1) * RW], in_=out_ps,
                        func=AF.Identity, bias=br_val,
                    )
                nc.sync.dma_start(out=t_out[:], in_=out_row)

            if repeat == 1:
                emit_body()
            else:
                with tc.For_i(0, repeat, 1):
                    emit_body()

    nc.finalize()
    return nc


def kernel(
    query_emb,
    entity_emb,
    neighbor_embs,
    neighbor_scores,
    Wp,
    bp,
    Wbil,
    bbil,
    Wg,
    g_bias,
    Wr,
    br,
):
    br_val = float(np.asarray(br).reshape(-1)[0])
    if "nc" not in _CACHE:
        _CACHE["nc"] = build_program(br_val)
    nc = _CACHE["nc"]

    bf = ml_dtypes.bfloat16
    q = np.asarray(query_emb, np.float32)
    ent = np.asarray(entity_emb, np.float32)
    nbr = np.asarray(neighbor_embs, np.float32)
    sc = np.asarray(neighbor_scores, np.float32)
    Wg_ = np.asarray(Wg, np.float32)
    Wbil_ = np.asarray(Wbil, np.float32)

    # ---- shared weight prep ----
    # Wtil[p,i,j] = sum_o Wg[p,o] Wbil[o,i,j]; W2til[i, p*D+j] = Wtil[p,i,j]
    Wtil = np.einsum("po,oij->pij", Wg_, Wbil_)
    W2til_f = Wtil.transpose(1, 0, 2).reshape(D, OJ)
    bbilg = Wg_ @ np.asarray(bbil, np.float32)  # [50]
    WpT_aug = np.zeros((EA, D), np.float32)
    WpT_aug[0:E] = np.asarray(Wp, np.float32).T
    WpT_aug[E] = np.asarray(bp, np.float32)
    # WgK[(db,k), c*D+o] = Wg[o, 2c+db]
    WgT = Wg_.T  # [d, o]
    WgK = np.empty((CR, NCH * D), np.float32)
    for c in range(NCH):
        WgK[:, c * D : (c + 1) * D] = np.repeat(WgT[2 * c : 2 * c + 2], K, axis=0)
    # wpack: WpT0 | WpT1 | bbilg_rep+WrT | WpT2
    wpack = np.zeros((P, 100 + D + 1 + D), np.float32)
    wpack[:, 0:D] = WpT_aug[0:128]
    wpack[:, D : 2 * D] = WpT_aug[128:256]
    wpack[:, 100:150] = bbilg[None, :]
    wpack[0:D, 150] = np.asarray(Wr, np.float32).reshape(-1)
    wpack[0 : EA - 256, 151 : 151 + D] = WpT_aug[256:EA]
    wpack = wpack.astype(bf)
    gb = np.asarray(g_bias, np.float32)

    in_maps = []
    for c in range(N_CORES):
        s = slice(c * NLOC, (c + 1) * NLOC)
        ent_aug = np.zeros((EA, NLOC), np.float32)
        ent_aug[0:E] = ent[s].T
        ent_aug[E] = 1.0
        nbrT = nbr[s].transpose(2, 1, 0).reshape(NCH * CR, NLOC)
        sT = sc[s, 0:K].T  # [63, NLOC]
        f32p = np.zeros((SR, NLOC // SR + 1), np.float32)
        f32p[:, 0 : NLOC // SR] = sc[s, K].reshape(NLOC // SR, SR).T
        f32p[0:D, NLOC // SR] = gb
        s63r = sc[s, K][None, :]  # [1, NLOC]
        in_maps.append(
            {
                "nbrT": np.ascontiguousarray(nbrT).astype(bf),
                "sWgK": np.ascontiguousarray(
                    np.concatenate(
                        [np.concatenate([sT, sT], axis=0), WgK], axis=1
                    )
                ).astype(bf),
                "f32p": np.ascontiguousarray(f32p),
                "s63r": np.ascontiguousarray(s63r).astype(bf),
                "qW": np.ascontiguousarray(
                    np.concatenate([q[s].T, W2til_f], axis=1)
                ).astype(bf),
                "entT0": np.ascontiguousarray(ent_aug[0:128]).astype(bf),
                "entT1": np.ascontiguousarray(ent_aug[128:256]).astype(bf),
                "entT2": np.ascontiguousarray(ent_aug[256:EA]).astype(bf),
                "wpack": wpack,
            }
        )

    _CACHE["last_in_maps"] = in_maps
    res = run_bass_kernel_spmd(nc, in_maps, core_ids=list(range(N_CORES)))
    out = np.concatenate(
        [res.results[c]["out"].reshape(NLOC, 1) for c in range(N_CORES)], axis=0
    )
    return out.astype(np.float32)



# revision 13
# speedup vs baseline: 133.5918x; 1.0041x over previous
"""Trainium2 Bass kernel for NeuralECMModel (gnn_message_passing).

Math (per node n):
  ent  = entity_emb @ Wp.T + bp                                   [N,50]
  node = einsum('ni,oij,nj->no', q, Wbil, ent) + bbil             [N,50]
  wtext= sum_k s[n,k]*nbr[n,k,:] + s[n,63]*node[n,:]              [N,50]
  agg  = wtext @ Wg.T                                             [N,50]
  out  = elu(agg + g_bias) @ Wr.T + br                            [N,1]

Key restructuring (vs naive): Wg is folded into both branches so `agg`
is accumulated directly in PSUM by the PE:
  agg[p,n] = sum_{(d,k)} Wg[p,d]*s[n,k]*nbr[n,k,d]        (PE contraction
             over 25 chunks of the transposed neighbor stream)
           + s63[n]*(q Wtil[p] ent + bbilg[p])            (row-major bilinear,
             transpose-matmul-accumulated into the same PSUM tile)
  with Wtil[p,i,j] = sum_o Wg[p,o]*Wbil[o,i,j], bbilg = Wg @ bbil.

This removes the k-tree reduction from the vector engines entirely; the
score multiply is ONE full-rate bf16 DVE op per 500-node macro tile.

Sharding: pure data parallel over nodes, N=20000 -> 2500 nodes/core x 8.
"""

import numpy as np
import ml_dtypes

import concourse.bass as bass
import concourse.bacc as bacc
import concourse.tile as tile
import concourse.mybir as mybir
from concourse.bass_utils import run_bass_kernel_spmd
from concourse.masks import make_identity

F32 = mybir.dt.float32
BF16 = mybir.dt.bfloat16
OP = mybir.AluOpType
AF = mybir.ActivationFunctionType
AX = mybir.AxisListType

N_CORES = 8
N = 20000
NLOC = N // N_CORES   # 2500
K = 63
D = 50
E = 300
EA = 304              # padded augmented entity rows (300 + ones + 3 zero)
P = 128
SR = 125              # bilinear sub-tile rows
NCH = 25              # neighbor (d,k) chunks of 126 rows
CR = 2 * K            # 126 rows per chunk (2 d's x 63 k's)
OJ = D * D            # 2500

RW = 500              # agg region width (one PSUM bank: 500 f32)
# per-sub-tile U o-chunk assignment (n_direct_psum_dve, n_sbuf_dve, n_pool)
UPLAN = ((0, 5, 0),)   # all U chunks: ACT egress copy -> DVE mul+reduce
# snbr chunk engine: True -> Pool, False -> DVE
SNBR_POOL = tuple(False for _ in range(25))
# U psum chunk width in o's (each *D wide); one PSUM bank per chunk so every
# matmul output is bank-aligned
UW = 10

_CACHE = {}


def _tree_levels(w):
    """Pairwise-halving splits: [(hi, lo), ...] meaning x[0:hi] += x[lo:lo+hi]."""
    out = []
    while w > 1:
        lo = (w + 1) // 2
        hi = w - lo
        out.append((hi, lo))
        w = lo
    return out


def build_program(br_val: float, skip_bil=False, skip_nbr=False, dump_ng=False,
                  repeat=1, uplan=None, dve_tree=False, snbr_pool=None):
    """repeat>1 wraps the whole per-run body (including all input streaming)
    in a hardware loop - used by test.py to time steady-state per-execution
    HW time with the ~3.6ms axon launch round-trip amortized away.

    v2 structure (single pass over all NLOC nodes):
      - neighbor stream DMA'd chunk-major: 25 chunks of [126, 2500] with 5KB
        contiguous rows (vs 0.5-1KB rows of the macro-tile slicing).
      - agg accumulated in 5 persistent PSUM bank regions [50, 500] covering
        all 2500 nodes; ONE tail (elu + Wr head) at the end.
      - bilinear j-reduction on DVE via a single tensor_reduce(X) instead of
        a 6-instruction pairwise tree; Pool keeps the tree for its o-share.
      - per-sub-tile engine assignment of the 5 U o-chunks: chunk 0 is
        consumed by DVE straight from PSUM (saves the ACT egress copy),
        the rest split DVE/Pool via an alternating pattern.
    """
    nc = bacc.Bacc("TRN2", debug=False, num_devices=N_CORES)

    # ---- per-core inputs (layouts unchanged from v1) ----
    t_nbrT = nc.dram_tensor("nbrT", [NCH * CR, NLOC], BF16, kind="ExternalInput")
    t_f32p = nc.dram_tensor("f32p", [SR, NLOC // SR + 1], F32, kind="ExternalInput")
    t_qW = nc.dram_tensor("qW", [D, NLOC + OJ], BF16, kind="ExternalInput")
    t_entT0 = nc.dram_tensor("entT0", [128, NLOC], BF16, kind="ExternalInput")
    t_entT1 = nc.dram_tensor("entT1", [128, NLOC], BF16, kind="ExternalInput")
    t_entT2 = nc.dram_tensor("entT2", [EA - 256, NLOC], BF16, kind="ExternalInput")
    t_wpack = nc.dram_tensor("wpack", [128, 100 + D + 1 + D], BF16, kind="ExternalInput")
    t_sWgK = nc.dram_tensor("sWgK", [CR, NLOC + NCH * D], BF16, kind="ExternalInput")
    t_s63r = nc.dram_tensor("s63r", [1, NLOC], BF16, kind="ExternalInput")
    t_out = nc.dram_tensor("out", [1, NLOC], F32, kind="ExternalOutput")

    NREG = NLOC // RW          # 5 agg regions of 500 nodes (1 PSUM bank each)
    NSUB = NLOC // SR          # 20 bilinear sub-tiles
    SPR = RW // SR             # 4 sub-tiles per region

    with tile.TileContext(nc) as tc:
        with (
            tc.tile_pool(name="res", bufs=1) as res,
            tc.tile_pool(name="nbrp", bufs=6) as nbrp,
            tc.tile_pool(name="snbrp", bufs=4) as snbrp,
            tc.tile_pool(name="usbd", bufs=3) as usbd_p,
            tc.tile_pool(name="usbg", bufs=3) as usbg_p,
            tc.tile_pool(name="prodd", bufs=3) as prodd_p,
            tc.tile_pool(name="prodg", bufs=3) as prodg_p,
            tc.tile_pool(name="small", bufs=8) as small,
            tc.tile_pool(name="nodegs", bufs=20) as nodegs,
            tc.tile_pool(name="tailp", bufs=3) as tailp,
            tc.tile_pool(name="ps_agg", bufs=1, space="PSUM") as ps_agg,
            tc.tile_pool(name="ps_u", bufs=2, space="PSUM") as ps_u,
            tc.tile_pool(name="ps_misc", bufs=1, space="PSUM") as ps_misc,
            nc.allow_low_precision(reason="bf16 bilinear reduce; node term is ~10% of signal"),
        ):
            ident_sb = res.tile([P, P], BF16)
            make_identity(nc, ident_sb)

            def emit_body():
                # ---- residents: sWgK first (feeds snbr_0, DVE's first op) ----
                sWgK_sb = res.tile([CR, NLOC + NCH * D], BF16)
                nc.sync.dma_start(out=sWgK_sb, in_=t_sWgK[:])
                qW_sb = res.tile([D, NLOC + OJ], BF16)
                nc.sync.dma_start(out=qW_sb, in_=t_qW[:])
                wpack_sb = res.tile([P, 100 + D + 1 + D], BF16)
                nc.sync.dma_start(out=wpack_sb, in_=t_wpack[:])
                f32p_sb = res.tile([SR, NLOC // SR + 1], F32)
                nc.sync.dma_start(out=f32p_sb, in_=t_f32p[:])
                entT_sb = [
                    res.tile([128, NLOC], BF16, tag="entT0", name="entT0_sb"),
                    res.tile([128, NLOC], BF16, tag="entT1", name="entT1_sb"),
                    res.tile([EA - 256, NLOC], BF16, tag="entT2", name="entT2_sb"),
                ]
                for sb, t in zip(entT_sb, (t_entT0, t_entT1, t_entT2)):
                    nc.sync.dma_start(out=sb, in_=t[:])
                s63r_sb = res.tile([1, NLOC], BF16)
                nc.sync.dma_start(out=s63r_sb, in_=t_s63r[:])
                out_row = res.tile([1, NLOC], F32)

                sTrep_sb = sWgK_sb[:, 0:NLOC]
                WgK_sb = sWgK_sb[:, NLOC : NLOC + NCH * D]
                qT_sb = qW_sb[:, 0:NLOC]
                W2til_sb = qW_sb[:, NLOC : NLOC + OJ]
                WpT_sb = [
                    wpack_sb[:, 0:D],
                    wpack_sb[:, D : 2 * D],
                    wpack_sb[0 : EA - 256, 151 : 151 + D],
                ]
                bbilg_rep = wpack_sb[:, 100:150]
                WrT_col = wpack_sb[0:D, 150:151]
                s63p_sb = f32p_sb[:, 0 : NLOC // SR]
                gb_sb = f32p_sb[0:D, NLOC // SR : NLOC // SR + 1]

                # ---- neighbor chunk DMAs (self-throttled by pool bufs) ----
                nbr_c = []
                for c in range(NCH if not skip_nbr else 0):
                    t = nbrp.tile([CR, NLOC], BF16, tag="nbr", name=f"nbr{c}")
                    nc.sync.dma_start(out=t, in_=t_nbrT[c * CR : (c + 1) * CR, :])
                    nbr_c.append(t)

                # persistent agg regions
                aggR = [
                    ps_agg.tile([D, RW], F32, tag=f"agg{r}", name=f"agg{r}")
                    for r in range(NREG)
                ]

                def emit_snbr(c):
                    st_ = snbrp.tile([CR, NLOC], BF16, tag="snbr", name=f"snbr{c}")
                    sp = snbr_pool if snbr_pool is not None else SNBR_POOL
                    eng = nc.gpsimd if sp[c] else nc.vector
                    eng.tensor_mul(st_, nbr_c[c], sTrep_sb)
                    return st_

                def emit_nbr_mm(c, snbr_t):
                    for r in range(NREG):
                        nc.tensor.matmul(
                            aggR[r],
                            WgK_sb[:, c * D : (c + 1) * D],
                            snbr_t[:, r * RW : (r + 1) * RW],
                            start=(c == 0),
                            stop=False,
                        )

                # ---- interleaved bilinear sub-tiles + neighbor chunks ----
                ci = 0  # next neighbor chunk to emit
                node_gbs = []

                def emit_chunk():
                    nonlocal ci
                    if not skip_nbr and ci < NCH:
                        emit_nbr_mm(ci, emit_snbr(ci))
                        ci += 1

                for st in range(NSUB if not skip_bil else 0):
                    r0 = st * SR
                    rs = slice(r0, r0 + SR)

                    ent_ps = ps_misc.tile([SR, D], F32, tag="misc", name="ent_ps")
                    for c in range(3):
                        nc.tensor.matmul(
                            ent_ps,
                            entT_sb[c][:, rs],
                            WpT_sb[c],
                            start=(c == 0),
                            stop=(c == 2),
                        )
                    ents = small.tile([SR, D], BF16, tag="ents")
                    nc.scalar.activation(
                        out=ents, in_=ent_ps, func=AF.Copy,
                        scale=s63p_sb[:, st : st + 1],
                    )
                    ev = ents.unsqueeze(1).broadcast_to([SR, D, D])

                    # U chunk plan for this sub-tile: (n_direct, n_dve, n_pool)
                    _up = uplan if uplan is not None else UPLAN
                    nD, nV, nG = _up[st % len(_up)]
                    node_g = nodegs.tile([SR, D], BF16, tag="node_g",
                                         name=f"node_g{st}")
                    node_gbs.append(node_g)

                    # direct-PSUM chunks: DVE consumes u_ps f32 (no ACT copy)
                    for k in range(nD):
                        o0 = k * UW
                        u_ps = ps_u.tile([SR, UW * D], F32, tag="ups")
                        nc.tensor.matmul(
                            u_ps, qT_sb[:, rs],
                            W2til_sb[:, o0 * D : (o0 + UW) * D],
                            start=True, stop=True,
                        )
                        prodd = prodd_p.tile([SR, UW * D], BF16, tag="prodD")
                        pv = prodd.rearrange("p (o j) -> p o j", o=UW)
                        nc.vector.tensor_mul(
                            pv, u_ps.rearrange("p (o j) -> p o j", o=UW),
                            ev[:, o0 : o0 + UW],
                        )
                        nc.vector.tensor_reduce(
                            node_g[:, o0 : o0 + UW].unsqueeze(2), pv,
                            axis=AX.X, op=OP.add,
                        )

                    # DVE-via-SBUF chunks (ACT egress copy, batched mul+reduce)
                    oV = nD * UW
                    wV = nV * UW
                    if nV:
                        usbd = usbd_p.tile([SR, wV * D], BF16, tag="usbd")
                        for k in range(nV):
                            o0 = (nD + k) * UW
                            u_ps = ps_u.tile([SR, UW * D], F32, tag="ups")
                            nc.tensor.matmul(
                                u_ps, qT_sb[:, rs],
                                W2til_sb[:, o0 * D : (o0 + UW) * D],
                                start=True, stop=True,
                            )
                            nc.scalar.copy(
                                out=usbd[:, k * UW * D : (k + 1) * UW * D],
                                in_=u_ps,
                            )
                        prodv = prodd_p.tile([SR, wV * D], BF16, tag="prodV")
                        pv = prodv.rearrange("p (o j) -> p o j", o=wV)
                        nc.vector.tensor_mul(
                            pv, usbd.rearrange("p (o j) -> p o j", o=wV),
                            ev[:, oV : oV + wV],
                        )
                        if dve_tree:
                            w = D
                            for hi, lo in _tree_levels(D):
                                if w == 2:
                                    nc.vector.tensor_add(
                                        node_g[:, oV : oV + wV].unsqueeze(2),
                                        pv[:, :, 0:1], pv[:, :, 1:2],
                                    )
                                else:
                                    nc.vector.tensor_add(
                                        pv[:, :, 0:hi], pv[:, :, 0:hi],
                                        pv[:, :, lo : lo + hi],
                                    )
                                w = lo
                        else:
                            nc.vector.tensor_reduce(
                                node_g[:, oV : oV + wV].unsqueeze(2), pv,
                                axis=AX.X, op=OP.add,
                            )

                    # Pool chunks (ACT egress copy, mul + pairwise tree)
                    oG = (nD + nV) * UW
                    wG = nG * UW
                    if nG:
                        usbg = usbg_p.tile([SR, wG * D], BF16, tag="usbg")
                        for k in range(nG):
                            o0 = (nD + nV + k) * UW
                            u_ps = ps_u.tile([SR, UW * D], F32, tag="ups")
                            nc.tensor.matmul(
                                u_ps, qT_sb[:, rs],
                                W2til_sb[:, o0 * D : (o0 + UW) * D],
                                start=True, stop=True,
                            )
                            nc.scalar.copy(
                                out=usbg[:, k * UW * D : (k + 1) * UW * D],
                                in_=u_ps,
                            )
                        prodg = prodg_p.tile([SR, wG * D], BF16, tag="prodG")
                        pg = prodg.rearrange("p (o j) -> p o j", o=wG)
                        nc.gpsimd.tensor_mul(
                            pg, usbg.rearrange("p (o j) -> p o j", o=wG),
                            ev[:, oG : oG + wG],
                        )
                        w = D
                        for hi, lo in _tree_levels(D):
                            if w == 2:
                                nc.gpsimd.tensor_add(
                                    node_g[:, oG : oG + wG].unsqueeze(2),
                                    pg[:, :, 0:1], pg[:, :, 1:2],
                                )
                            else:
                                nc.gpsimd.tensor_add(
                                    pg[:, :, 0:hi], pg[:, :, 0:hi],
                                    pg[:, :, lo : lo + hi],
                                )
                            w = lo

                    # keep the neighbor pipeline fed
                    emit_chunk()
                    if st % 2 == 1:
                        emit_chunk()

                while ci < NCH and not skip_nbr:
                    emit_chunk()

                # node contribution -> agg (transpose-accumulate), deferred
                # so chunk 0's start=True full-region write is always first
                for st, ng in enumerate(node_gbs):
                    r = st // SPR
                    nc.tensor.matmul(
                        aggR[r][:, (st % SPR) * SR : (st % SPR + 1) * SR],
                        ng,
                        ident_sb[0:SR, 0:SR],
                        start=(skip_nbr and st % SPR == 0),
                        stop=False,
                    )

                # rank-1 bias term closes each region's accumulation group
                for r in range(NREG):
                    nc.tensor.matmul(
                        aggR[r],
                        bbilg_rep[0:1],
                        s63r_sb[:, r * RW : (r + 1) * RW],
                        start=(skip_nbr and skip_bil),
                        stop=True,
                    )

                # ---- tail: elu(agg+gb) @ Wr + br, one pass per region ----
                for r in range(NREG):
                    e_sb = tailp.tile([D, RW], BF16, tag="e")
                    nc.scalar.activation(out=e_sb, in_=aggR[r], func=AF.Exp,
                                         bias=gb_sb)
                    r_sb = tailp.tile([D, RW], BF16, tag="r")
                    nc.scalar.activation(out=r_sb, in_=aggR[r], func=AF.Relu,
                                         bias=gb_sb)
                    feats = tailp.tile([D, RW], BF16, tag="feats")
                    nc.vector.scalar_tensor_tensor(
                        out=feats, in0=e_sb, scalar=-1.0, in1=r_sb,
                        op0=OP.add, op1=OP.min,
                    )
                    out_ps = ps_misc.tile([1, RW], F32, tag="misc", name="out_ps")
                    nc.tensor.matmul(out_ps, WrT_col, feats, start=True, stop=True)
                    nc.scalar.activation(
                        out=out_row[:, r * RW : (r + 1) * RW], in_=out_ps,
                        func=AF.Identity, bias=br_val,
                    )
                nc.sync.dma_start(out=t_out[:], in_=out_row)

            if repeat == 1:
                emit_body()
            else:
                with tc.For_i(0, repeat, 1):
                    emit_body()

    nc.finalize()
    return nc


def kernel(
    query_emb,
    entity_emb,
    neighbor_embs,
    neighbor_scores,
    Wp,
    bp,
    Wbil,
    bbil,
    Wg,
    g_bias,
    Wr,
    br,
):
    br_val = float(np.asarray(br).reshape(-1)[0])
    if "nc" not in _CACHE:
        _CACHE["nc"] = build_program(br_val)
    nc = _CACHE["nc"]

    bf = ml_dtypes.bfloat16
    q = np.asarray(query_emb, np.float32)
    ent = np.asarray(entity_emb, np.float32)
    nbr = np.asarray(neighbor_embs, np.float32)
    sc = np.asarray(neighbor_scores, np.float32)
    Wg_ = np.asarray(Wg, np.float32)
    Wbil_ = np.asarray(Wbil, np.float32)

    # ---- shared weight prep ----
    # Wtil[p,i,j] = sum_o Wg[p,o] Wbil[o,i,j]; W2til[i, p*D+j] = Wtil[p,i,j]
    Wtil = np.einsum("po,oij->pij", Wg_, Wbil_)
    W2til_f = Wtil.transpose(1, 0, 2).reshape(D, OJ)
    bbilg = Wg_ @ np.asarray(bbil, np.float32)  # [50]
    WpT_aug = np.zeros((EA, D), np.float32)
    WpT_aug[0:E] = np.asarray(Wp, np.float32).T
    WpT_aug[E] = np.asarray(bp, np.float32)
    # WgK[(db,k), c*D+o] = Wg[o, 2c+db]
    WgT = Wg_.T  # [d, o]
    WgK = np.empty((CR, NCH * D), np.float32)
    for c in range(NCH):
        WgK[:, c * D : (c + 1) * D] = np.repeat(WgT[2 * c : 2 * c + 2], K, axis=0)
    # wpack: WpT0 | WpT1 | bbilg_rep+WrT | WpT2
    wpack = np.zeros((P, 100 + D + 1 + D), np.float32)
    wpack[:, 0:D] = WpT_aug[0:128]
    wpack[:, D : 2 * D] = WpT_aug[128:256]
    wpack[:, 100:150] = bbilg[None, :]
    wpack[0:D, 150] = np.asarray(Wr, np.float32).reshape(-1)
    wpack[0 : EA - 256, 151 : 151 + D] = WpT_aug[256:EA]
    wpack = wpack.astype(bf)
    gb = np.asarray(g_bias, np.float32)

    in_maps = []
    for c in range(N_CORES):
        s = slice(c * NLOC, (c + 1) * NLOC)
        ent_aug = np.zeros((EA, NLOC), np.float32)
        ent_aug[0:E] = ent[s].T
        ent_aug[E] = 1.0
        nbrT = nbr[s].transpose(2, 1, 0).reshape(NCH * CR, NLOC)
        sT = sc[s, 0:K].T  # [63, NLOC]
        f32p = np.zeros((SR, NLOC // SR + 1), np.float32)
        f32p[:, 0 : NLOC // SR] = sc[s, K].reshape(NLOC // SR, SR).T
        f32p[0:D, NLOC // SR] = gb
        s63r = sc[s, K][None, :]  # [1, NLOC]
        in_maps.append(
            {
                "nbrT": np.ascontiguousarray(nbrT).astype(bf),
                "sWgK": np.ascontiguousarray(
                    np.concatenate(
                        [np.concatenate([sT, sT], axis=0), WgK], axis=1
                    )
                ).astype(bf),
                "f32p": np.ascontiguousarray(f32p),
                "s63r": np.ascontiguousarray(s63r).astype(bf),
                "qW": np.ascontiguousarray(
                    np.concatenate([q[s].T, W2til_f], axis=1)
                ).astype(bf),
                "entT0": np.ascontiguousarray(ent_aug[0:128]).astype(bf),
                "entT1": np.ascontiguousarray(ent_aug[128:256]).astype(bf),
                "entT2": np.ascontiguousarray(ent_aug[256:EA]).astype(bf),
                "wpack": wpack,
            }
        )

    _CACHE["last_in_maps"] = in_maps
    res = run_bass_kernel_spmd(nc, in_maps, core_ids=list(range(N_CORES)))
    out = np.concatenate(
        [res.results[c]["out"].reshape(NLOC, 1) for c in range(N_CORES)], axis=0
    )
    return out.astype(np.float32)

